# revision 9
# baseline (speedup 1.0000x reference)
"""Trainium2 Bass kernel for the NNConv/GRU/Set2Set message-passing network.

Strategy (8 NeuronCores, SPMD single program):
  - Nodes are padded to 10240 = 80 blocks of 128; core c owns blocks
    [10c, 10c+10) i.e. nodes [1280c, 1280(c+1)).
  - Edges are bucketed by destination block (host side); every
    (core, block) bucket is padded to a fixed K tiles of 128 edges so the
    device program is identical on all cores; only input data differs.
  - Per edge tile: PE recomputes the edge-conditioned 32x32 matrix
    ew = relu(ea @ W1 + b1) @ W2 (H^T block stationary, W2 streamed),
    DVE does msg[e,o] = sum_i x_src[e,i] * ew[e,i,o] via one broadcast
    multiply + one strided reduce, and PE scatter-adds [msg | x_src] into
    the block accumulator with a one-hot matmul (one-hot built on GPSIMD
    from iota == dstslot).  Because a core owns every edge that targets its
    blocks, aggregation completes locally - no AllReduce.
  - Node update (deg scaling, conv root, GRU) runs on the core's own 1280
    nodes in feature-major ("T") layout; updated features are transposed to
    row-major and AllGathered into the full [10240, 32] table, which is both
    the next iteration's gather source and the Set2Set input.
  - Set2Set (3 steps) + final linears run replicated on every core; core 0's
    output is returned.  The softmax skips the segment-max subtraction:
    |e| <= 32*max|h|*1 < ~64, exp(64) ~ 6e27 << f32 max, and the reference's
    max-subtracted form is algebraically identical.
"""
import sys
sys.path.insert(0, "/opt/trn_rl_repo")
import numpy as np
import concourse.bass as bass
import concourse.bacc as bacc
import concourse.tile as tile
import concourse.mybir as mybir
from concourse import bass_utils
from concourse.masks import make_identity

F32 = mybir.dt.float32
I32 = mybir.dt.int32
AF = mybir.ActivationFunctionType
OP = mybir.AluOpType
AX = mybir.AxisListType

NCORES = 8
P = 128
DIM = 32
KH = 128          # hidden dim of the edge MLP
N = 10000
E = 160000
B = 64
NBLK = 80         # node blocks of 128
NPAD = NBLK * P   # 10240
BPC = NBLK // NCORES   # blocks per core = 10
NLOC = BPC * P         # nodes per core = 1280

_PROGRAM_CACHE = {}


def _chunks(total, step):
    out = []
    s = 0
    while s < total:
        out.append((s, min(step, total - s)))
        s += step
    return out


def _build_program(K):
    """Build the SPMD Bass program for K edge-tiles per (core, block)."""
    T = BPC * K            # edge tiles per core
    ET = T * P             # padded edges per core

    nc = bacc.Bacc("TRN2", target_bir_lowering=False, debug=False,
                   enable_asserts=False, num_devices=NCORES)

    di = {}

    def inp(name, shape, dtype=F32):
        di[name] = nc.dram_tensor(name, shape, dtype, kind="ExternalInput")
        return di[name]

    ea_t = inp("ea_t", [2, ET])
    src_i = inp("src_i", [P, T], I32)
    dst_f = inp("dst_f", [P, T])
    iota_in = inp("iota_in", [P, P])
    w2p_in = inp("w2p", [KH, 1024])
    w1_in = inp("w1", [2, KH])
    b1_in = inp("b1", [KH, 1])
    lin0aug_in = inp("lin0aug", [2, DIM])
    xaug_in = inp("xaug", [2, NPAD])
    xaug_loc_in = inp("xaug_loc", [2, NLOC])
    deginv_in = inp("deginv", [DIM, NLOC])
    b2_in = inp("b2", [DIM, DIM])
    cr_in = inp("cr", [DIM, DIM])
    cb_in = inp("cb", [DIM, 1])
    gwih_in = inp("gwih", [DIM, 3 * DIM])
    gwhh_in = inp("gwhh", [DIM, 3 * DIM])
    gbr_in = inp("gbr", [DIM, 1])
    gbz_in = inp("gbz", [DIM, 1])
    gbin_in = inp("gbin", [DIM, 1])
    gbhn_in = inp("gbhn", [DIM, 1])
    ohb_in = inp("ohb", [P, NBLK * B])
    ohbt_in = inp("ohbt", [B, NBLK * P])
    lwih_in = inp("lwih", [2 * DIM, 4 * DIM])
    lwhh_in = inp("lwhh", [DIM, 4 * DIM])
    lb_in = inp("lb", [4 * DIM, 1])
    l1w_in = inp("l1w", [2 * DIM, DIM])
    l1b_in = inp("l1b", [DIM, 1])
    l2w_in = inp("l2w", [DIM, 1])
    l2b_in = inp("l2b", [1, 1])

    y_out = nc.dram_tensor("y", [1, B], F32, kind="ExternalOutput")

    ht_dram = nc.dram_tensor("ht_scr", [KH, ET], F32, kind="Internal")
    rm0 = nc.dram_tensor("rm0", [NPAD, DIM], F32, kind="Internal")
    ccin = [nc.dram_tensor(f"ccin{i}", [NLOC, DIM], F32, kind="Internal")
            for i in range(3)]
    ccout = [nc.dram_tensor(f"ccout{i}", [NPAD, DIM], F32, kind="Internal",
                            addr_space="Shared") for i in range(3)]
    rgroups = [list(range(NCORES))]

    with tile.TileContext(nc) as tc:
        with tc.tile_pool(name="pc", bufs=1) as pc, \
             tc.tile_pool(name="pht", bufs=4) as pht, \
             tc.tile_pool(name="poh", bufs=4) as poh, \
             tc.tile_pool(name="pmsg", bufs=4) as pmsg, \
             tc.tile_pool(name="ptmp", bufs=4) as ptmp, \
             tc.tile_pool(name="pnt", bufs=8) as pnt, \
             tc.tile_pool(name="pxa", bufs=2) as pxa, \
             tc.tile_pool(name="pbig", bufs=1) as pbig, \
             tc.tile_pool(name="pew", bufs=3, space="PSUM") as pew, \
             tc.tile_pool(name="pagg", bufs=2, space="PSUM") as pagg, \
             tc.tile_pool(name="pnode", bufs=3, space="PSUM") as pnp:

            def sconst(name, src, shape, dtype=F32):
                t = pc.tile(shape, dtype, tag=name)
                nc.sync.dma_start(out=t[:], in_=src[:])
                return t

            ident = pc.tile([P, P], F32, tag="ident")
            make_identity(nc, ident[:])
            iota = sconst("iota", iota_in, [P, P])
            w2p = sconst("w2p", w2p_in, [KH, 1024])
            w1 = sconst("w1", w1_in, [2, KH])
            b1 = sconst("b1", b1_in, [KH, 1])
            lin0aug = sconst("lin0aug", lin0aug_in, [2, DIM])
            xaug_loc = sconst("xaug_loc", xaug_loc_in, [2, NLOC])
            deginv = sconst("deginv", deginv_in, [DIM, NLOC])
            b2 = sconst("b2", b2_in, [DIM, DIM])
            cr = sconst("cr", cr_in, [DIM, DIM])
            cb = sconst("cb", cb_in, [DIM, 1])
            gwih = sconst("gwih", gwih_in, [DIM, 3 * DIM])
            gwhh = sconst("gwhh", gwhh_in, [DIM, 3 * DIM])
            gbr = sconst("gbr", gbr_in, [DIM, 1])
            gbz = sconst("gbz", gbz_in, [DIM, 1])
            gbin = sconst("gbin", gbin_in, [DIM, 1])
            gbhn = sconst("gbhn", gbhn_in, [DIM, 1])
            srcI = sconst("srcI", src_i, [P, T], I32)
            dstF = sconst("dstF", dst_f, [P, T])
            lwih = sconst("lwih", lwih_in, [2 * DIM, 4 * DIM])
            lwhh = sconst("lwhh", lwhh_in, [DIM, 4 * DIM])
            lb = sconst("lb", lb_in, [4 * DIM, 1])
            l1w = sconst("l1w", l1w_in, [2 * DIM, DIM])
            l1b = sconst("l1b", l1b_in, [DIM, 1])
            l2w = sconst("l2w", l2w_in, [DIM, 1])
            l2b = sconst("l2b", l2b_in, [1, 1])

            # ---- setup: H^T = relu(W1^T @ ea^T + b1), stored to DRAM ----
            for (s, w) in _chunks(ET, 512):
                eat = pnt.tile([2, 512], F32, tag="nt")
                nc.sync.dma_start(out=eat[:, :w], in_=ea_t[:, s:s + w])
                hp = pnp.tile([KH, 512], F32, tag="node")
                nc.tensor.matmul(out=hp[:, :w], lhsT=w1[:], rhs=eat[:, :w],
                                 start=True, stop=True)
                hs = pnt.tile([KH, 512], F32, tag="nt")
                nc.scalar.activation(out=hs[:, :w], in_=hp[:, :w],
                                     func=AF.Relu, bias=b1[:, :1])
                nc.sync.dma_start(out=ht_dram[:, s:s + w], in_=hs[:, :w])

            # ---- setup: out0 row-major table rm0 = relu(x @ W0 + b0) ----
            for b in range(NBLK):
                xab = pxa.tile([2, P], F32, tag="xab")
                nc.sync.dma_start(out=xab[:], in_=xaug_in[:, b * P:(b + 1) * P])
                op0 = pnp.tile([P, DIM], F32, tag="node")
                nc.tensor.matmul(out=op0[:], lhsT=xab[:],
                                 rhs=lin0aug[:], start=True, stop=True)
                os0 = pmsg.tile([P, DIM], F32, tag="os0")
                nc.scalar.activation(out=os0[:], in_=op0[:], func=AF.Relu)
                nc.sync.dma_start(out=rm0[b * P:(b + 1) * P, :], in_=os0[:])

            # ---- setup: hT_loc = out0^T for own nodes ----
            hT = pbig.tile([DIM, NLOC], F32, tag="hT0")
            for (s, w) in _chunks(NLOC, 512):
                hp0 = pnp.tile([DIM, 512], F32, tag="node")
                nc.tensor.matmul(out=hp0[:, :w], lhsT=lin0aug[:],
                                 rhs=xaug_loc[:, s:s + w], start=True, stop=True)
                nc.scalar.activation(out=hT[:, s:s + w], in_=hp0[:, :w], func=AF.Relu)

            # ---- 3 message-passing iterations ----
            for it in range(3):
                tab = rm0 if it == 0 else ccout[it - 1]
                aggM = pbig.tile([DIM, NLOC], F32, tag=f"aggM{it % 2}")
                aggS = pbig.tile([DIM, NLOC], F32, tag=f"aggS{it % 2}")
                for g in range(BPC):
                    aggp = pagg.tile([P, 2 * DIM], F32, tag="agg")
                    for j in range(K):
                        t = g * K + j
                        htt = pht.tile([KH, P], F32, tag="htt")
                        nc.sync.dma_start(out=htt[:], in_=ht_dram[:, t * P:(t + 1) * P])
                        msgx = pmsg.tile([P, 2 * DIM], F32, tag="msgx")
                        nc.gpsimd.indirect_dma_start(
                            out=msgx[:, DIM:2 * DIM], out_offset=None,
                            in_=tab[:],
                            in_offset=bass.IndirectOffsetOnAxis(ap=srcI[:, t:t + 1], axis=0))
                        oh = poh.tile([P, P], F32, tag="oh")
                        nc.gpsimd.tensor_scalar(out=oh[:], in0=iota[:],
                                                scalar1=dstF[:, t:t + 1], scalar2=None,
                                                op0=OP.is_equal)
                        for h in range(2):
                            ewp = pew.tile([P, 512], F32, tag="ew")
                            nc.tensor.matmul(out=ewp[:], lhsT=htt[:],
                                             rhs=w2p[:, h * 512:(h + 1) * 512],
                                             start=True, stop=True)
                            tmp = ptmp.tile([P, 512], F32, tag="tmp")
                            nc.vector.tensor_tensor(
                                out=tmp[:].rearrange("p (o i) -> p o i", i=DIM),
                                in0=ewp[:].rearrange("p (o i) -> p o i", i=DIM),
                                in1=msgx[:, DIM:2 * DIM].unsqueeze(1).to_broadcast([P, 16, DIM]),
                                op=OP.mult)
                            nc.vector.reduce_sum(
                                out=msgx[:, h * 16:(h + 1) * 16].unsqueeze(2),
                                in_=tmp[:].rearrange("p (o i) -> p o i", i=DIM),
                                axis=AX.X)
                        nc.tensor.matmul(out=aggp[:], lhsT=oh[:], rhs=msgx[:],
                                         start=(j == 0), stop=(j == K - 1))
                    # block g done -> transpose msg/x halves to T space (base 0)
                    aggs = pmsg.tile([P, 2 * DIM], F32, tag="aggs")
                    nc.scalar.activation(out=aggs[:], in_=aggp[:], func=AF.Copy)
                    trm = pnp.tile([DIM, P], F32, tag="node")
                    nc.tensor.transpose(out=trm[:], in_=aggs[:, 0:DIM], identity=ident[:])
                    nc.scalar.activation(out=aggM[:, g * P:(g + 1) * P], in_=trm[:],
                                         func=AF.Copy)
                    trs = pnp.tile([DIM, P], F32, tag="node")
                    nc.tensor.transpose(out=trs[:], in_=aggs[:, DIM:2 * DIM],
                                        identity=ident[:])
                    nc.scalar.activation(out=aggS[:, g * P:(g + 1) * P], in_=trs[:],
                                         func=AF.Copy)

                # ---- node update on own 1280 nodes (T layout) ----
                hTn = pbig.tile([DIM, NLOC], F32, tag=f"hTn{it % 2}")
                mT = pbig.tile([DIM, NLOC], F32, tag="mT")
                for (s, w) in _chunks(NLOC, 512):
                    pm = pnp.tile([DIM, 512], F32, tag="node")
                    nc.tensor.matmul(out=pm[:, :w], lhsT=b2[:],
                                     rhs=aggS[:, s:s + w], start=True, stop=False)
                    nc.tensor.matmul(out=pm[:, :w], lhsT=ident[0:DIM, 0:DIM],
                                     rhs=aggM[:, s:s + w], start=False, stop=True)
                    t1 = pnt.tile([DIM, 512], F32, tag="nt")
                    nc.vector.tensor_tensor(out=t1[:, :w], in0=pm[:, :w],
                                            in1=deginv[:, s:s + w], op=OP.mult)
                    pm2 = pnp.tile([DIM, 512], F32, tag="node")
                    nc.tensor.matmul(out=pm2[:, :w], lhsT=cr[:], rhs=hT[:, s:s + w],
                                     start=True, stop=True)
                    t2 = pnt.tile([DIM, 512], F32, tag="nt")
                    nc.vector.tensor_add(out=t2[:, :w], in0=t1[:, :w], in1=pm2[:, :w])
                    nc.scalar.activation(out=mT[:, s:s + w], in_=t2[:, :w],
                                         func=AF.Relu, bias=cb[:, :1])
                    # GRU
                    prz = pnp.tile([2 * DIM, 512], F32, tag="node")
                    nc.tensor.matmul(out=prz[:, :w], lhsT=gwih[:, 0:2 * DIM],
                                     rhs=mT[:, s:s + w], start=True, stop=False)
                    nc.tensor.matmul(out=prz[:, :w], lhsT=gwhh[:, 0:2 * DIM],
                                     rhs=hT[:, s:s + w], start=False, stop=True)
                    rg = pnt.tile([DIM, 512], F32, tag="nt")
                    nc.scalar.activation(out=rg[:, :w], in_=prz[0:DIM, :w],
                                         func=AF.Sigmoid, bias=gbr[:, :1])
                    zg = pnt.tile([DIM, 512], F32, tag="nt")
                    nc.scalar.activation(out=zg[:, :w], in_=prz[DIM:2 * DIM, :w],
                                         func=AF.Sigmoid, bias=gbz[:, :1])
                    pin = pnp.tile([DIM, 512], F32, tag="node")
                    nc.tensor.matmul(out=pin[:, :w], lhsT=gwih[:, 2 * DIM:3 * DIM],
                                     rhs=mT[:, s:s + w], start=True, stop=True)
                    phn = pnp.tile([DIM, 512], F32, tag="node")
                    nc.tensor.matmul(out=phn[:, :w], lhsT=gwhh[:, 2 * DIM:3 * DIM],
                                     rhs=hT[:, s:s + w], start=True, stop=True)
                    hn = pnt.tile([DIM, 512], F32, tag="nt")
                    nc.scalar.activation(out=hn[:, :w], in_=phn[:, :w],
                                         func=AF.Identity, bias=gbhn[:, :1])
                    t3 = pnt.tile([DIM, 512], F32, tag="nt")
                    nc.vector.tensor_tensor(out=t3[:, :w], in0=rg[:, :w],
                                            in1=hn[:, :w], op=OP.mult)
                    t4 = pnt.tile([DIM, 512], F32, tag="nt")
                    nc.vector.tensor_add(out=t4[:, :w], in0=t3[:, :w], in1=pin[:, :w])
                    ng = pnt.tile([DIM, 512], F32, tag="nt")
                    nc.scalar.activation(out=ng[:, :w], in_=t4[:, :w],
                                         func=AF.Tanh, bias=gbin[:, :1])
                    t5 = pnt.tile([DIM, 512], F32, tag="nt")
                    nc.vector.tensor_sub(out=t5[:, :w], in0=hT[:, s:s + w], in1=ng[:, :w])
                    t6 = pnt.tile([DIM, 512], F32, tag="nt")
                    nc.vector.tensor_tensor(out=t6[:, :w], in0=t5[:, :w],
                                            in1=zg[:, :w], op=OP.mult)
                    nc.vector.tensor_add(out=hTn[:, s:s + w], in0=ng[:, :w], in1=t6[:, :w])

                # ---- transpose own nodes to row-major and AllGather ----
                for g in range(BPC):
                    trp2 = pnp.tile([P, DIM], F32, tag="node")
                    nc.tensor.transpose(out=trp2[:], in_=hTn[:, g * P:(g + 1) * P],
                                        identity=ident[0:DIM, 0:DIM])
                    trs2 = pmsg.tile([P, DIM], F32, tag="trs2")
                    nc.scalar.activation(out=trs2[:], in_=trp2[:], func=AF.Copy)
                    nc.sync.dma_start(out=ccin[it][g * P:(g + 1) * P, :], in_=trs2[:])
                nc.gpsimd.collective_compute(
                    "AllGather", OP.bypass, replica_groups=rgroups,
                    ins=[ccin[it][:]], outs=[ccout[it][:]])
                hT = hTn

            # ---- Set2Set pooling (replicated on every core) ----
            rmsb = pbig.tile([P, NBLK * DIM], F32, tag="rmsb")
            for b in range(NBLK):
                nc.sync.dma_start(out=rmsb[:, b * DIM:(b + 1) * DIM],
                                  in_=ccout[2][b * P:(b + 1) * P, :])
            qstar = pc.tile([2 * DIM, B], F32, tag="qstar")
            hh = pc.tile([DIM, B], F32, tag="hh")
            ccs = pc.tile([DIM, B], F32, tag="ccs")
            nc.gpsimd.memset(qstar[:], 0.0)
            nc.gpsimd.memset(hh[:], 0.0)
            nc.gpsimd.memset(ccs[:], 0.0)
            QG = pbig.tile([P, NBLK * DIM], F32, tag="QG")
            prod = pbig.tile([P, NBLK * DIM], F32, tag="prod")
            aout = pbig.tile([P, NBLK * DIM], F32, tag="aout")
            for step in range(3):
                gp = pnp.tile([4 * DIM, B], F32, tag="node")
                nc.tensor.matmul(out=gp[:], lhsT=lwih[:], rhs=qstar[:],
                                 start=True, stop=False)
                nc.tensor.matmul(out=gp[:], lhsT=lwhh[:], rhs=hh[:],
                                 start=False, stop=True)
                gi = pnt.tile([DIM, B], F32, tag="nt")
                gf = pnt.tile([DIM, B], F32, tag="nt")
                gg = pnt.tile([DIM, B], F32, tag="nt")
                go = pnt.tile([DIM, B], F32, tag="nt")
                nc.scalar.activation(out=gi[:], in_=gp[0:DIM, :], func=AF.Sigmoid,
                                     bias=lb[0:DIM, :1])
                nc.scalar.activation(out=gf[:], in_=gp[DIM:2 * DIM, :], func=AF.Sigmoid,
                                     bias=lb[DIM:2 * DIM, :1])
                nc.scalar.activation(out=gg[:], in_=gp[2 * DIM:3 * DIM, :], func=AF.Tanh,
                                     bias=lb[2 * DIM:3 * DIM, :1])
                nc.scalar.activation(out=go[:], in_=gp[3 * DIM:4 * DIM, :], func=AF.Sigmoid,
                                     bias=lb[3 * DIM:4 * DIM, :1])
                u1 = pnt.tile([DIM, B], F32, tag="nt")
                nc.vector.tensor_tensor(out=u1[:], in0=gf[:], in1=ccs[:], op=OP.mult)
                u2 = pnt.tile([DIM, B], F32, tag="nt")
                nc.vector.tensor_tensor(out=u2[:], in0=gi[:], in1=gg[:], op=OP.mult)
                nc.vector.tensor_add(out=ccs[:], in0=u1[:], in1=u2[:])
                tcc = pnt.tile([DIM, B], F32, tag="nt")
                nc.scalar.activation(out=tcc[:], in_=ccs[:], func=AF.Tanh)
                nc.vector.tensor_tensor(out=hh[:], in0=go[:], in1=tcc[:], op=OP.mult)
                # q = hh; build q in graph-major layout [B, DIM]
                qgp = pnp.tile([B, DIM], F32, tag="node")
                nc.tensor.transpose(out=qgp[:], in_=hh[:], identity=ident[0:DIM, 0:DIM])
                qg64 = pnt.tile([B, DIM], F32, tag="nt")
                nc.scalar.activation(out=qg64[:], in_=qgp[:], func=AF.Copy)
                # QG[n, :] = q[batch[n], :] per block
                for b in range(NBLK):
                    ohbt_t = pxa.tile([B, P], F32, tag="ohbt_t")
                    nc.sync.dma_start(out=ohbt_t[:], in_=ohbt_in[:, b * P:(b + 1) * P])
                    qb = pnp.tile([P, DIM], F32, tag="node")
                    nc.tensor.matmul(out=qb[:], lhsT=ohbt_t[:],
                                     rhs=qg64[:], start=True, stop=True)
                    nc.scalar.activation(out=QG[:, b * DIM:(b + 1) * DIM], in_=qb[:],
                                         func=AF.Copy)
                # e[n] = <out[n], q[batch[n]]>;  a = exp(e) (no max; bounded)
                nc.vector.tensor_tensor(out=prod[:], in0=rmsb[:], in1=QG[:], op=OP.mult)
                esb = pnt.tile([P, NBLK], F32, tag="nt")
                nc.vector.reduce_sum(
                    out=esb[:].unsqueeze(2),
                    in_=prod[:].rearrange("p (b o) -> p b o", o=DIM),
                    axis=AX.X)
                asb = pnt.tile([P, NBLK], F32, tag="nt")
                nc.scalar.activation(out=asb[:], in_=esb[:], func=AF.Exp)
                nc.vector.tensor_tensor(
                    out=aout[:].rearrange("p (b o) -> p b o", o=DIM),
                    in0=rmsb[:].rearrange("p (b o) -> p b o", o=DIM),
                    in1=asb[:].unsqueeze(2).to_broadcast([P, NBLK, DIM]),
                    op=OP.mult)
                rvp = pagg.tile([B, DIM], F32, tag="agg")
                asp = pagg.tile([B, 1], F32, tag="agg")
                for b in range(NBLK):
                    ohb_t = pxa.tile([P, B], F32, tag="ohb_t")
                    nc.sync.dma_start(out=ohb_t[:], in_=ohb_in[:, b * B:(b + 1) * B])
                    nc.tensor.matmul(out=rvp[:], lhsT=ohb_t[:],
                                     rhs=aout[:, b * DIM:(b + 1) * DIM],
                                     start=(b == 0), stop=(b == NBLK - 1))
                    nc.tensor.matmul(out=asp[:], lhsT=ohb_t[:],
                                     rhs=asb[:, b:b + 1],
                                     start=(b == 0), stop=(b == NBLK - 1))
                ras = pnt.tile([B, 1], F32, tag="nt")
                nc.vector.reciprocal(out=ras[:], in_=asp[:])
                rvs = pnt.tile([B, DIM], F32, tag="nt")
                nc.vector.tensor_scalar(out=rvs[:], in0=rvp[:], scalar1=ras[:, :1],
                                        scalar2=None, op0=OP.mult)
                rvtp = pnp.tile([DIM, B], F32, tag="node")
                nc.tensor.transpose(out=rvtp[:], in_=rvs[:], identity=ident[0:B, 0:B])
                nc.scalar.activation(out=qstar[0:DIM, :], in_=hh[:], func=AF.Copy)
                nc.scalar.activation(out=qstar[DIM:2 * DIM, :], in_=rvtp[:], func=AF.Copy)

            # ---- final linears: y = relu(qstar^T @ l1 + b) @ l2 + b ----
            y1p = pnp.tile([DIM, B], F32, tag="node")
            nc.tensor.matmul(out=y1p[:], lhsT=l1w[:], rhs=qstar[:], start=True, stop=True)
            y1s = pnt.tile([DIM, B], F32, tag="nt")
            nc.scalar.activation(out=y1s[:], in_=y1p[:], func=AF.Relu, bias=l1b[:, :1])
            y2p = pnp.tile([1, B], F32, tag="node")
            nc.tensor.matmul(out=y2p[:], lhsT=l2w[:], rhs=y1s[:], start=True, stop=True)
            y2s = pnt.tile([1, B], F32, tag="nt")
            nc.scalar.activation(out=y2s[:], in_=y2p[:], func=AF.Identity, bias=l2b[:, :1])
            nc.sync.dma_start(out=y_out[:], in_=y2s[:])

    nc.compile()
    return nc


def _prep_inputs(inputs):
    x = np.asarray(inputs["x"], np.float32)
    edge_attr = np.asarray(inputs["edge_attr"], np.float32)
    edge_index = np.asarray(inputs["edge_index"]).astype(np.int64)
    batch = np.asarray(inputs["batch"]).astype(np.int64)
    src = edge_index[0]
    dst = edge_index[1]

    deg = np.maximum(np.bincount(dst, minlength=N), 1).astype(np.float32)
    deginv_full = np.zeros(NPAD, np.float32)
    deginv_full[:N] = 1.0 / deg
    deginv_full[N:] = 1.0

    order = np.argsort(dst, kind="stable")
    dsts = dst[order]
    srcs = src[order]
    eas = edge_attr[order]
    blk_of = (dsts >> 7).astype(np.int64)
    cnt = np.bincount(blk_of, minlength=NBLK)
    K = int(np.ceil(cnt.max() / P))
    T = BPC * K
    EPB = K * P  # padded edges per block

    starts = np.zeros(NBLK + 1, np.int64)
    starts[1:] = np.cumsum(cnt)
    pos = np.arange(E, dtype=np.int64) - starts[blk_of]
    slot = blk_of * EPB + pos

    p_src = np.zeros(NBLK * EPB, np.int32)
    p_dstf = np.full(NBLK * EPB, 999.0, np.float32)
    p_ea = np.zeros((NBLK * EPB, 2), np.float32)
    p_src[slot] = srcs.astype(np.int32)
    p_dstf[slot] = (dsts & 127).astype(np.float32)
    p_ea[slot] = eas

    iota_np = np.broadcast_to(np.arange(P, dtype=np.float32), (P, P)).copy()
    w2 = np.asarray(inputs["nn_w2"], np.float32)
    w2p = w2.reshape(KH, DIM, DIM).transpose(0, 2, 1).reshape(KH, 1024).copy()
    lin0aug = np.concatenate([np.asarray(inputs["lin0_w"], np.float32),
                              np.asarray(inputs["lin0_b"], np.float32)[None, :]], 0)
    xaug = np.zeros((2, NPAD), np.float32)
    xaug[0, :N] = x[:, 0]
    xaug[1, :N] = 1.0

    gih = np.asarray(inputs["gru_b_ih"], np.float32)
    ghh = np.asarray(inputs["gru_b_hh"], np.float32)

    ohb_np = np.zeros((P, NBLK, B), np.float32)
    ohbt_np = np.zeros((B, NBLK, P), np.float32)
    nidx = np.arange(N)
    ohb_np[nidx & 127, nidx >> 7, batch] = 1.0
    ohbt_np[batch, nidx >> 7, nidx & 127] = 1.0

    common = {
        "iota_in": iota_np,
        "w2p": w2p,
        "w1": np.asarray(inputs["nn_w1"], np.float32),
        "b1": np.asarray(inputs["nn_b1"], np.float32)[:, None].copy(),
        "lin0aug": lin0aug,
        "xaug": xaug,
        "b2": np.asarray(inputs["nn_b2"], np.float32).reshape(DIM, DIM).copy(),
        "cr": np.asarray(inputs["conv_root"], np.float32),
        "cb": np.asarray(inputs["conv_bias"], np.float32)[:, None].copy(),
        "gwih": np.asarray(inputs["gru_w_ih"], np.float32),
        "gwhh": np.asarray(inputs["gru_w_hh"], np.float32),
        "gbr": (gih + ghh)[0:DIM, None].copy(),
        "gbz": (gih + ghh)[DIM:2 * DIM, None].copy(),
        "gbin": gih[2 * DIM:3 * DIM, None].copy(),
        "gbhn": ghh[2 * DIM:3 * DIM, None].copy(),
        "ohb": ohb_np.reshape(P, NBLK * B),
        "ohbt": ohbt_np.reshape(B, NBLK * P),
        "lwih": np.asarray(inputs["lstm_w_ih"], np.float32),
        "lwhh": np.asarray(inputs["lstm_w_hh"], np.float32),
        "lb": (np.asarray(inputs["lstm_b_ih"], np.float32)
               + np.asarray(inputs["lstm_b_hh"], np.float32))[:, None].copy(),
        "l1w": np.asarray(inputs["lin1_w"], np.float32),
        "l1b": np.asarray(inputs["lin1_b"], np.float32)[:, None].copy(),
        "l2w": np.asarray(inputs["lin2_w"], np.float32),
        "l2b": np.asarray(inputs["lin2_b"], np.float32)[:, None].copy(),
    }

    in_maps = []
    for c in range(NCORES):
        lo = c * BPC * EPB
        hi = (c + 1) * BPC * EPB
        m = dict(common)
        m["ea_t"] = np.ascontiguousarray(p_ea[lo:hi].T)
        m["src_i"] = np.ascontiguousarray(p_src[lo:hi].reshape(T, P).T)
        m["dst_f"] = np.ascontiguousarray(p_dstf[lo:hi].reshape(T, P).T)
        m["xaug_loc"] = np.ascontiguousarray(xaug[:, c * NLOC:(c + 1) * NLOC])
        m["deginv"] = np.broadcast_to(deginv_full[c * NLOC:(c + 1) * NLOC],
                                      (DIM, NLOC)).copy()
        in_maps.append(m)
    return in_maps, K


def kernel(**inputs):
    in_maps, K = _prep_inputs(inputs)
    if K not in _PROGRAM_CACHE:
        _PROGRAM_CACHE[K] = _build_program(K)
    nc = _PROGRAM_CACHE[K]
    res = bass_utils.run_bass_kernel_spmd(nc, in_maps, core_ids=list(range(NCORES)))
    return np.asarray(res.results[0]["y"][0], np.float32)


# revision 20
# speedup vs baseline: 3.3507x; 3.3507x over previous
"""Trainium2 Bass kernel for the NNConv/GRU/Set2Set message-passing network.

Strategy (8 NeuronCores, SPMD single program):
  - Nodes are padded to 10240 = 80 blocks of 128; core c owns blocks
    [10c, 10c+10) i.e. nodes [1280c, 1280(c+1)).
  - Edges are bucketed by destination block (host side); every
    (core, block) bucket is padded to a fixed K tiles of 128 edges so the
    device program is identical on all cores; only input data differs.
  - Per edge tile: PE recomputes the edge-conditioned 32x32 matrix
    ew = relu(ea @ W1 + b1) @ W2 (H^T block stationary, W2 streamed),
    DVE does msg[e,o] = sum_i x_src[e,i] * ew[e,i,o] via one broadcast
    multiply + one strided reduce, and PE scatter-adds [msg | x_src] into
    the block accumulator with a one-hot matmul (one-hot built on GPSIMD
    from iota == dstslot).  Because a core owns every edge that targets its
    blocks, aggregation completes locally - no AllReduce.
  - Node update (deg scaling, conv root, GRU) runs on the core's own 1280
    nodes in feature-major ("T") layout; updated features are transposed to
    row-major and AllGathered into the full [10240, 32] table, which is both
    the next iteration's gather source and the Set2Set input.
  - Set2Set (3 steps) + final linears run replicated on every core; core 0's
    output is returned.  The softmax skips the segment-max subtraction:
    |e| <= 32*max|h|*1 < ~64, exp(64) ~ 6e27 << f32 max, and the reference's
    max-subtracted form is algebraically identical.
"""
import sys
sys.path.insert(0, "/opt/trn_rl_repo")
import numpy as np
import concourse.bass as bass
import concourse.bacc as bacc
import concourse.tile as tile
import concourse.mybir as mybir
from concourse import bass_utils
from concourse.masks import make_identity

F32 = mybir.dt.float32
F32R = mybir.dt.float32r
I32 = mybir.dt.int32
AF = mybir.ActivationFunctionType
OP = mybir.AluOpType
AX = mybir.AxisListType

NCORES = 8
P = 128
DIM = 32
KH = 128          # hidden dim of the edge MLP
N = 10000
E = 160000
B = 64
NBLK = 80         # node blocks of 128
NPAD = NBLK * P   # 10240
BPC = NBLK // NCORES   # blocks per core = 10
NLOC = BPC * P         # nodes per core = 1280

_PROGRAM_CACHE = {}
ABLATE = set()  # profiling-only: names of stages to skip in _build_program


def _chunks(total, step):
    out = []
    s = 0
    while s < total:
        out.append((s, min(step, total - s)))
        s += step
    return out


def _build_program(K, single=False):
    """Build the SPMD Bass program for K edge-tiles per (core, block).

    single=True builds a 1-core variant with the AllGather replaced by a
    local DMA (wrong data, right cost) for TimelineSim profiling.
    """
    T = BPC * K            # edge tiles per core
    ET = T * P             # padded edges per core

    nc = bacc.Bacc("TRN2", target_bir_lowering=False, debug=False,
                   enable_asserts=False,
                   num_devices=1 if single else NCORES)

    di = {}

    def inp(name, shape, dtype=F32):
        di[name] = nc.dram_tensor(name, shape, dtype, kind="ExternalInput")
        return di[name]

    ea_t = inp("ea_t", [2, ET])
    src_i = inp("src_i", [P, T], I32)
    dst_f = inp("dst_f", [P, T])
    iota_in = inp("iota_in", [P, P])
    w2p_in = inp("w2p", [KH, 1024])
    w1_in = inp("w1", [2, KH])
    b1_in = inp("b1", [KH, 1])
    lin0aug_in = inp("lin0aug", [2, DIM])
    xaug_in = inp("xaug", [2, NPAD])
    xaug_loc_in = inp("xaug_loc", [2, NLOC])
    deginv_in = inp("deginv", [DIM, NLOC])
    b2_in = inp("b2", [DIM, DIM])
    cr_in = inp("cr", [DIM, DIM])
    cb_in = inp("cb", [DIM, 1])
    gwih_in = inp("gwih", [DIM, 3 * DIM])
    gwhh_in = inp("gwhh", [DIM, 3 * DIM])
    gbr_in = inp("gbr", [DIM, 1])
    gbz_in = inp("gbz", [DIM, 1])
    gbin_in = inp("gbin", [DIM, 1])
    gbhn_in = inp("gbhn", [DIM, 1])
    ohb_in = inp("ohb", [P, NBLK * B])
    ohbt_in = inp("ohbt", [B, NBLK * P])
    lwih_in = inp("lwih", [2 * DIM, 4 * DIM])
    lwhh_in = inp("lwhh", [DIM, 4 * DIM])
    lb_in = inp("lb", [4 * DIM, 1])
    l1w_in = inp("l1w", [2 * DIM, DIM])
    l1b_in = inp("l1b", [DIM, 1])
    l2w_in = inp("l2w", [DIM, 1])
    l2b_in = inp("l2b", [1, 1])

    y_out = nc.dram_tensor("y", [1, B], F32, kind="ExternalOutput")

    ht_dram = nc.dram_tensor("ht_scr", [KH, ET], F32R, kind="Internal")
    oh_dram = nc.dram_tensor("oh_scr", [P, T * P], F32R, kind="Internal")
    rm0 = nc.dram_tensor("rm0", [NPAD, DIM], F32, kind="Internal")
    ccin = [nc.dram_tensor(f"ccin{i}", [NLOC, DIM], F32, kind="Internal")
            for i in range(3)]
    ccout = [nc.dram_tensor(f"ccout{i}", [NPAD, DIM], F32, kind="Internal",
                            addr_space="Shared") for i in range(3)]
    rgroups = [list(range(NCORES))]

    with tile.TileContext(nc) as tc:
        with tc.tile_pool(name="pc", bufs=1) as pc, \
             tc.tile_pool(name="pht", bufs=2) as pht, \
             tc.tile_pool(name="poh", bufs=4) as poh, \
             tc.tile_pool(name="pmsg", bufs=4) as pmsg, \
             tc.tile_pool(name="ptmp", bufs=3) as ptmp, \
             tc.tile_pool(name="pnt", bufs=8) as pnt, \
             tc.tile_pool(name="pxa", bufs=2) as pxa, \
             tc.tile_pool(name="pbig", bufs=1) as pbig, \
             tc.tile_pool(name="pxg", bufs=6) as pxg, \
             tc.tile_pool(name="pew", bufs=3, space="PSUM") as pew, \
             tc.tile_pool(name="pagg", bufs=1, space="PSUM") as pagg, \
             tc.tile_pool(name="pnode", bufs=2, space="PSUM") as pnp:

            def sconst(name, src, shape, dtype=F32):
                t = pc.tile(shape, dtype, tag=name)
                nc.sync.dma_start(out=t[:], in_=src[:])
                return t

            ident = pc.tile([P, P], F32, tag="ident")
            make_identity(nc, ident[:])
            iota = sconst("iota", iota_in, [P, P])
            w2p = sconst("w2p", w2p_in, [KH, 1024])
            w2r = pc.tile([KH, 1024], F32R, tag="w2r")
            nc.vector.tensor_copy(out=w2r[:], in_=w2p[:])
            w1 = sconst("w1", w1_in, [2, KH])
            b1 = sconst("b1", b1_in, [KH, 1])
            lin0aug = sconst("lin0aug", lin0aug_in, [2, DIM])
            xaug_loc = sconst("xaug_loc", xaug_loc_in, [2, NLOC])
            deginv = sconst("deginv", deginv_in, [DIM, NLOC])
            b2 = sconst("b2", b2_in, [DIM, DIM])
            cr = sconst("cr", cr_in, [DIM, DIM])
            cb = sconst("cb", cb_in, [DIM, 1])
            gwih = sconst("gwih", gwih_in, [DIM, 3 * DIM])
            gwhh = sconst("gwhh", gwhh_in, [DIM, 3 * DIM])
            gbr = sconst("gbr", gbr_in, [DIM, 1])
            gbz = sconst("gbz", gbz_in, [DIM, 1])
            gbin = sconst("gbin", gbin_in, [DIM, 1])
            gbhn = sconst("gbhn", gbhn_in, [DIM, 1])
            srcI = sconst("srcI", src_i, [P, T], I32)
            dstF = sconst("dstF", dst_f, [P, T])
            lwih = sconst("lwih", lwih_in, [2 * DIM, 4 * DIM])
            lwhh = sconst("lwhh", lwhh_in, [DIM, 4 * DIM])
            lb = sconst("lb", lb_in, [4 * DIM, 1])
            l1w = sconst("l1w", l1w_in, [2 * DIM, DIM])
            l1b = sconst("l1b", l1b_in, [DIM, 1])
            l2w = sconst("l2w", l2w_in, [DIM, 1])
            l2b = sconst("l2b", l2b_in, [1, 1])

            # ---- setup: H^T = relu(W1^T @ ea^T + b1) -> f32r, stored to DRAM ----
            for (s, w) in _chunks(ET, 1024):
                eat = pxa.tile([2, 1024], F32, tag="eat")
                nc.sync.dma_start(out=eat[:, :w], in_=ea_t[:, s:s + w])
                hs = pht.tile([KH, 1024], F32R, tag="hs")
                for (s2, w2) in _chunks(w, 512):
                    hp = pnp.tile([KH, 512], F32, tag="node")
                    nc.tensor.matmul(out=hp[:, :w2], lhsT=w1[:],
                                     rhs=eat[:, s2:s2 + w2], start=True, stop=True)
                    nc.scalar.activation(out=hs[:, s2:s2 + w2], in_=hp[:, :w2],
                                         func=AF.Relu, bias=b1[:, :1])
                nc.sync.dma_start(out=ht_dram[:, s:s + w], in_=hs[:, :w])


            # ---- setup: out0 row-major table rm0 = relu(x @ W0 + b0) ----
            for b in range(NBLK):
                xab = pxa.tile([2, P], F32, tag="xab")
                nc.scalar.dma_start(out=xab[:], in_=xaug_in[:, b * P:(b + 1) * P])
                op0 = pnp.tile([P, DIM], F32, tag="node")
                nc.tensor.matmul(out=op0[:], lhsT=xab[:],
                                 rhs=lin0aug[:], start=True, stop=True)
                os0 = pmsg.tile([P, DIM], F32, tag="os0")
                nc.scalar.activation(out=os0[:], in_=op0[:], func=AF.Relu)
                nc.sync.dma_start(out=rm0[b * P:(b + 1) * P, :], in_=os0[:])

            # ---- setup: hT_loc = out0^T for own nodes ----
            hT = pbig.tile([DIM, NLOC], F32, tag="hTn1")
            for (s, w) in _chunks(NLOC, 512):
                hp0 = pnp.tile([DIM, 512], F32, tag="node")
                nc.tensor.matmul(out=hp0[:, :w], lhsT=lin0aug[:],
                                 rhs=xaug_loc[:, s:s + w], start=True, stop=True)
                nc.scalar.activation(out=hT[:, s:s + w], in_=hp0[:, :w], func=AF.Relu)

            # ---- 3 message-passing iterations ----
            for it in range(3):
                tab = rm0 if it == 0 else ccout[it - 1]
                aggM = pbig.tile([DIM, NLOC], F32, tag="aggM")
                aggS = pbig.tile([DIM, NLOC], F32, tag="aggS")
                for g in range(BPC):
                    htg = pht.tile([KH, K * P], F32R, tag="htg")
                    nc.scalar.dma_start(out=htg[:],
                                        in_=ht_dram[:, g * K * P:(g + 1) * K * P])
                    ohg = pht.tile([P, K * P], F32R, tag="ohg2")
                    if it == 0:
                        for j in range(K):
                            nc.gpsimd.tensor_scalar(
                                out=ohg[:, j * P:(j + 1) * P], in0=iota[:],
                                scalar1=dstF[:, g * K + j:g * K + j + 1], scalar2=None,
                                op0=OP.is_equal)
                        nc.scalar.dma_start(out=oh_dram[:, g * K * P:(g + 1) * K * P],
                                            in_=ohg[:])
                    else:
                        nc.scalar.dma_start(out=ohg[:],
                                            in_=oh_dram[:, g * K * P:(g + 1) * K * P])
                    # wide accumulator: [msg-decomposed (o,i) | x] per node slot
                    aggw = pagg.tile([P, 1056], F32, tag="aggw")
                    for j in range(K):
                        t = g * K + j
                        xg = pxg.tile([P, DIM], F32, tag="xg")
                        nc.gpsimd.indirect_dma_start(
                            out=xg[:], out_offset=None, in_=tab[:],
                            in_offset=bass.IndirectOffsetOnAxis(ap=srcI[:, t:t + 1], axis=0))
                        tmp = ptmp.tile([P, 1056], F32R, tag="tmp")
                        for h in range(2):
                            ewp = pew.tile([P, 512], F32, tag="ew")
                            nc.tensor.matmul(out=ewp[:],
                                             lhsT=htg[:, j * P:(j + 1) * P],
                                             rhs=w2r[:, h * 512:(h + 1) * 512],
                                             start=True, stop=True)
                            nc.vector.tensor_tensor(
                                out=tmp[:, h * 512:(h + 1) * 512].rearrange(
                                    "p (o i) -> p o i", i=DIM),
                                in0=ewp[:].rearrange("p (o i) -> p o i", i=DIM),
                                in1=xg[:].unsqueeze(1).to_broadcast([P, 16, DIM]),
                                op=OP.mult)
                        nc.scalar.activation(out=tmp[:, 1024:1056], in_=xg[:], func=AF.Copy)
                        st = (j == 0)
                        sp = (j == K - 1)
                        nc.tensor.matmul(out=aggw[:, 0:512], lhsT=ohg[:, j * P:(j + 1) * P],
                                         rhs=tmp[:, 0:512], start=st, stop=sp)
                        nc.tensor.matmul(out=aggw[:, 512:1024], lhsT=ohg[:, j * P:(j + 1) * P],
                                         rhs=tmp[:, 512:1024], start=st, stop=sp)
                        nc.tensor.matmul(out=aggw[:, 1024:1056], lhsT=ohg[:, j * P:(j + 1) * P],
                                         rhs=tmp[:, 1024:1056], start=st, stop=sp)
                    # block g done: fold the i-sum, split msg/x, transpose to T space
                    aggms = pmsg.tile([P, DIM], F32, tag="aggms")
                    nc.vector.reduce_sum(
                        out=aggms[:].unsqueeze(2),
                        in_=aggw[:, 0:1024].rearrange("p (o i) -> p o i", i=DIM),
                        axis=AX.X)
                    aggxs = pmsg.tile([P, DIM], F32, tag="aggxs")
                    nc.scalar.activation(out=aggxs[:], in_=aggw[:, 1024:1056], func=AF.Copy)
                    trm = pnp.tile([DIM, P], F32, tag="node")
                    nc.tensor.transpose(out=trm[:], in_=aggms[:], identity=ident[:])
                    nc.scalar.activation(out=aggM[:, g * P:(g + 1) * P], in_=trm[:],
                                         func=AF.Copy)
                    trs = pnp.tile([DIM, P], F32, tag="node")
                    nc.tensor.transpose(out=trs[:], in_=aggxs[:], identity=ident[:])
                    nc.scalar.activation(out=aggS[:, g * P:(g + 1) * P], in_=trs[:],
                                         func=AF.Copy)

                # ---- node update on own 1280 nodes (T layout) ----
                hTn = pbig.tile([DIM, NLOC], F32, tag=f"hTn{it % 2}")
                mT = pbig.tile([DIM, NLOC], F32, tag="mT")
                for (s, w) in _chunks(NLOC, 512):
                    pm = pnp.tile([DIM, 512], F32, tag="node")
                    nc.tensor.matmul(out=pm[:, :w], lhsT=b2[:],
                                     rhs=aggS[:, s:s + w], start=True, stop=False)
                    nc.tensor.matmul(out=pm[:, :w], lhsT=ident[0:DIM, 0:DIM],
                                     rhs=aggM[:, s:s + w], start=False, stop=True)
                    t1 = pnt.tile([DIM, 512], F32, tag="nt")
                    nc.vector.tensor_tensor(out=t1[:, :w], in0=pm[:, :w],
                                            in1=deginv[:, s:s + w], op=OP.mult)
                    pm2 = pnp.tile([DIM, 512], F32, tag="node")
                    nc.tensor.matmul(out=pm2[:, :w], lhsT=cr[:], rhs=hT[:, s:s + w],
                                     start=True, stop=True)
                    t2 = pnt.tile([DIM, 512], F32, tag="nt")
                    nc.vector.tensor_add(out=t2[:, :w], in0=t1[:, :w], in1=pm2[:, :w])
                    nc.scalar.activation(out=mT[:, s:s + w], in_=t2[:, :w],
                                         func=AF.Relu, bias=cb[:, :1])
                    # GRU
                    prz = pnp.tile([2 * DIM, 512], F32, tag="node")
                    nc.tensor.matmul(out=prz[:, :w], lhsT=gwih[:, 0:2 * DIM],
                                     rhs=mT[:, s:s + w], start=True, stop=False)
                    nc.tensor.matmul(out=prz[:, :w], lhsT=gwhh[:, 0:2 * DIM],
                                     rhs=hT[:, s:s + w], start=False, stop=True)
                    rg = pnt.tile([DIM, 512], F32, tag="nt")
                    nc.scalar.activation(out=rg[:, :w], in_=prz[0:DIM, :w],
                                         func=AF.Sigmoid, bias=gbr[:, :1])
                    zg = pnt.tile([DIM, 512], F32, tag="nt")
                    nc.scalar.activation(out=zg[:, :w], in_=prz[DIM:2 * DIM, :w],
                                         func=AF.Sigmoid, bias=gbz[:, :1])
                    pin = pnp.tile([DIM, 512], F32, tag="node")
                    nc.tensor.matmul(out=pin[:, :w], lhsT=gwih[:, 2 * DIM:3 * DIM],
                                     rhs=mT[:, s:s + w], start=True, stop=True)
                    phn = pnp.tile([DIM, 512], F32, tag="node")
                    nc.tensor.matmul(out=phn[:, :w], lhsT=gwhh[:, 2 * DIM:3 * DIM],
                                     rhs=hT[:, s:s + w], start=True, stop=True)
                    hn = pnt.tile([DIM, 512], F32, tag="nt")
                    nc.scalar.activation(out=hn[:, :w], in_=phn[:, :w],
                                         func=AF.Identity, bias=gbhn[:, :1])
                    t3 = pnt.tile([DIM, 512], F32, tag="nt")
                    nc.vector.tensor_tensor(out=t3[:, :w], in0=rg[:, :w],
                                            in1=hn[:, :w], op=OP.mult)
                    t4 = pnt.tile([DIM, 512], F32, tag="nt")
                    nc.vector.tensor_add(out=t4[:, :w], in0=t3[:, :w], in1=pin[:, :w])
                    ng = pnt.tile([DIM, 512], F32, tag="nt")
                    nc.scalar.activation(out=ng[:, :w], in_=t4[:, :w],
                                         func=AF.Tanh, bias=gbin[:, :1])
                    t5 = pnt.tile([DIM, 512], F32, tag="nt")
                    nc.vector.tensor_sub(out=t5[:, :w], in0=hT[:, s:s + w], in1=ng[:, :w])
                    t6 = pnt.tile([DIM, 512], F32, tag="nt")
                    nc.vector.tensor_tensor(out=t6[:, :w], in0=t5[:, :w],
                                            in1=zg[:, :w], op=OP.mult)
                    nc.vector.tensor_add(out=hTn[:, s:s + w], in0=ng[:, :w], in1=t6[:, :w])

                # ---- transpose own nodes to row-major and AllGather ----
                for g in range(BPC):
                    trp2 = pnp.tile([P, DIM], F32, tag="node")
                    nc.tensor.transpose(out=trp2[:], in_=hTn[:, g * P:(g + 1) * P],
                                        identity=ident[0:DIM, 0:DIM])
                    trs2 = pmsg.tile([P, DIM], F32, tag="trs2")
                    nc.scalar.activation(out=trs2[:], in_=trp2[:], func=AF.Copy)
                    nc.sync.dma_start(out=ccin[it][g * P:(g + 1) * P, :], in_=trs2[:])
                if single:
                    nc.gpsimd.dma_start(out=ccout[it][0:NLOC, :], in_=ccin[it][:])
                else:
                    nc.gpsimd.collective_compute(
                        "AllGather", OP.bypass, replica_groups=rgroups,
                        ins=[ccin[it][:]], outs=[ccout[it][:]])
                hT = hTn

            # ---- Set2Set pooling (replicated on every core) ----
            rmsb = pbig.tile([P, NBLK * DIM], F32, tag="rmsb")
            for b in range(NBLK):
                nc.sync.dma_start(out=rmsb[:, b * DIM:(b + 1) * DIM],
                                  in_=ccout[2][b * P:(b + 1) * P, :])
            qstar = pc.tile([2 * DIM, B], F32, tag="qstar")
            hh = pc.tile([DIM, B], F32, tag="hh")
            ccs = pc.tile([DIM, B], F32, tag="ccs")
            nc.gpsimd.memset(qstar[:], 0.0)
            nc.gpsimd.memset(hh[:], 0.0)
            nc.gpsimd.memset(ccs[:], 0.0)
            QG = pbig.tile([P, NBLK * DIM], F32, tag="QG")
            prod = pbig.tile([P, NBLK * DIM], F32, tag="prod")
            aout = pbig.tile([P, NBLK * DIM], F32, tag="aout")
            for step in range(3):
                gp = pew.tile([4 * DIM, B], F32, tag="ew")
                nc.tensor.matmul(out=gp[:], lhsT=lwih[:], rhs=qstar[:],
                                 start=True, stop=False)
                nc.tensor.matmul(out=gp[:], lhsT=lwhh[:], rhs=hh[:],
                                 start=False, stop=True)
                gi = pnt.tile([DIM, B], F32, tag="nt")
                gf = pnt.tile([DIM, B], F32, tag="nt")
                gg = pnt.tile([DIM, B], F32, tag="nt")
                go = pnt.tile([DIM, B], F32, tag="nt")
                nc.scalar.activation(out=gi[:], in_=gp[0:DIM, :], func=AF.Sigmoid,
                                     bias=lb[0:DIM, :1])
                nc.scalar.activation(out=gf[:], in_=gp[DIM:2 * DIM, :], func=AF.Sigmoid,
                                     bias=lb[DIM:2 * DIM, :1])
                nc.scalar.activation(out=gg[:], in_=gp[2 * DIM:3 * DIM, :], func=AF.Tanh,
                                     bias=lb[2 * DIM:3 * DIM, :1])
                nc.scalar.activation(out=go[:], in_=gp[3 * DIM:4 * DIM, :], func=AF.Sigmoid,
                                     bias=lb[3 * DIM:4 * DIM, :1])
                u1 = pnt.tile([DIM, B], F32, tag="nt")
                nc.vector.tensor_tensor(out=u1[:], in0=gf[:], in1=ccs[:], op=OP.mult)
                u2 = pnt.tile([DIM, B], F32, tag="nt")
                nc.vector.tensor_tensor(out=u2[:], in0=gi[:], in1=gg[:], op=OP.mult)
                nc.vector.tensor_add(out=ccs[:], in0=u1[:], in1=u2[:])
                tcc = pnt.tile([DIM, B], F32, tag="nt")
                nc.scalar.activation(out=tcc[:], in_=ccs[:], func=AF.Tanh)
                nc.vector.tensor_tensor(out=hh[:], in0=go[:], in1=tcc[:], op=OP.mult)
                # q = hh; build q in graph-major layout [B, DIM]
                qgp = pnp.tile([B, DIM], F32, tag="node")
                nc.tensor.transpose(out=qgp[:], in_=hh[:], identity=ident[0:DIM, 0:DIM])
                qg64 = pnt.tile([B, DIM], F32, tag="nt")
                nc.scalar.activation(out=qg64[:], in_=qgp[:], func=AF.Copy)
                # QG[n, :] = q[batch[n], :] per block
                for cb_ in range(NBLK // 16):
                    ohbt_c = pxa.tile([B, 16 * P], F32, tag="ohbt_c")
                    nc.scalar.dma_start(out=ohbt_c[:],
                                        in_=ohbt_in[:, cb_ * 16 * P:(cb_ + 1) * 16 * P])
                    qgc = pew.tile([P, 512], F32, tag="ew")
                    for j in range(16):
                        nc.tensor.matmul(out=qgc[:, j * DIM:(j + 1) * DIM],
                                         lhsT=ohbt_c[:, j * P:(j + 1) * P],
                                         rhs=qg64[:], start=True, stop=True)
                    nc.scalar.activation(out=QG[:, cb_ * 512:(cb_ + 1) * 512],
                                         in_=qgc[:], func=AF.Copy)
                # e[n] = <out[n], q[batch[n]]>;  a = exp(e) (no max; bounded)
                nc.vector.tensor_tensor(out=prod[:], in0=rmsb[:], in1=QG[:], op=OP.mult)
                esb = pnt.tile([P, NBLK], F32, tag="nt")
                nc.vector.reduce_sum(
                    out=esb[:].unsqueeze(2),
                    in_=prod[:].rearrange("p (b o) -> p b o", o=DIM),
                    axis=AX.X)
                asb = pnt.tile([P, NBLK], F32, tag="nt")
                nc.scalar.activation(out=asb[:], in_=esb[:], func=AF.Exp)
                nc.vector.tensor_tensor(
                    out=aout[:].rearrange("p (b o) -> p b o", o=DIM),
                    in0=rmsb[:].rearrange("p (b o) -> p b o", o=DIM),
                    in1=asb[:].unsqueeze(2).to_broadcast([P, NBLK, DIM]),
                    op=OP.mult)
                rvp = pagg.tile([B, DIM], F32, tag="aggw")
                asp = pnp.tile([B, 1], F32, tag="node")
                for cb_ in range(NBLK // 16):
                    ohb_c = pxa.tile([P, 16 * B], F32, tag="ohb_c")
                    nc.sync.dma_start(out=ohb_c[:],
                                      in_=ohb_in[:, cb_ * 16 * B:(cb_ + 1) * 16 * B])
                    for j in range(16):
                        b = cb_ * 16 + j
                        nc.tensor.matmul(out=rvp[:], lhsT=ohb_c[:, j * B:(j + 1) * B],
                                         rhs=aout[:, b * DIM:(b + 1) * DIM],
                                         start=(b == 0), stop=(b == NBLK - 1))
                        nc.tensor.matmul(out=asp[:], lhsT=ohb_c[:, j * B:(j + 1) * B],
                                         rhs=asb[:, b:b + 1],
                                         start=(b == 0), stop=(b == NBLK - 1))
                ras = pnt.tile([B, 1], F32, tag="nt")
                nc.vector.reciprocal(out=ras[:], in_=asp[:])
                rvs = pnt.tile([B, DIM], F32, tag="nt")
                nc.vector.tensor_scalar(out=rvs[:], in0=rvp[:], scalar1=ras[:, :1],
                                        scalar2=None, op0=OP.mult)
                rvtp = pnp.tile([DIM, B], F32, tag="node")
                nc.tensor.transpose(out=rvtp[:], in_=rvs[:], identity=ident[0:B, 0:B])
                nc.scalar.activation(out=qstar[0:DIM, :], in_=hh[:], func=AF.Copy)
                nc.scalar.activation(out=qstar[DIM:2 * DIM, :], in_=rvtp[:], func=AF.Copy)

            # ---- final linears: y = relu(qstar^T @ l1 + b) @ l2 + b ----
            y1p = pnp.tile([DIM, B], F32, tag="node")
            nc.tensor.matmul(out=y1p[:], lhsT=l1w[:], rhs=qstar[:], start=True, stop=True)
            y1s = pnt.tile([DIM, B], F32, tag="nt")
            nc.scalar.activation(out=y1s[:], in_=y1p[:], func=AF.Relu, bias=l1b[:, :1])
            y2p = pnp.tile([1, B], F32, tag="node")
            nc.tensor.matmul(out=y2p[:], lhsT=l2w[:], rhs=y1s[:], start=True, stop=True)
            y2s = pnt.tile([1, B], F32, tag="nt")
            nc.scalar.activation(out=y2s[:], in_=y2p[:], func=AF.Identity, bias=l2b[:, :1])
            nc.sync.dma_start(out=y_out[:], in_=y2s[:])

    nc.compile()
    return nc


def _prep_inputs(inputs):
    x = np.asarray(inputs["x"], np.float32)
    edge_attr = np.asarray(inputs["edge_attr"], np.float32)
    edge_index = np.asarray(inputs["edge_index"]).astype(np.int64)
    batch = np.asarray(inputs["batch"]).astype(np.int64)
    src = edge_index[0]
    dst = edge_index[1]

    deg = np.maximum(np.bincount(dst, minlength=N), 1).astype(np.float32)
    deginv_full = np.zeros(NPAD, np.float32)
    deginv_full[:N] = 1.0 / deg
    deginv_full[N:] = 1.0

    order = np.argsort(dst, kind="stable")
    dsts = dst[order]
    srcs = src[order]
    eas = edge_attr[order]
    blk_of = (dsts >> 7).astype(np.int64)
    cnt = np.bincount(blk_of, minlength=NBLK)
    K = int(np.ceil(cnt.max() / P))
    T = BPC * K
    EPB = K * P  # padded edges per block

    starts = np.zeros(NBLK + 1, np.int64)
    starts[1:] = np.cumsum(cnt)
    pos = np.arange(E, dtype=np.int64) - starts[blk_of]
    slot = blk_of * EPB + pos

    p_src = np.zeros(NBLK * EPB, np.int32)
    p_dstf = np.full(NBLK * EPB, 999.0, np.float32)
    p_ea = np.zeros((NBLK * EPB, 2), np.float32)
    p_src[slot] = srcs.astype(np.int32)
    p_dstf[slot] = (dsts & 127).astype(np.float32)
    p_ea[slot] = eas

    iota_np = np.broadcast_to(np.arange(P, dtype=np.float32), (P, P)).copy()
    w2 = np.asarray(inputs["nn_w2"], np.float32)
    w2p = w2.reshape(KH, DIM, DIM).transpose(0, 2, 1).reshape(KH, 1024).copy()
    lin0aug = np.concatenate([np.asarray(inputs["lin0_w"], np.float32),
                              np.asarray(inputs["lin0_b"], np.float32)[None, :]], 0)
    xaug = np.zeros((2, NPAD), np.float32)
    xaug[0, :N] = x[:, 0]
    xaug[1, :N] = 1.0

    gih = np.asarray(inputs["gru_b_ih"], np.float32)
    ghh = np.asarray(inputs["gru_b_hh"], np.float32)

    ohb_np = np.zeros((P, NBLK, B), np.float32)
    ohbt_np = np.zeros((B, NBLK, P), np.float32)
    nidx = np.arange(N)
    ohb_np[nidx & 127, nidx >> 7, batch] = 1.0
    ohbt_np[batch, nidx >> 7, nidx & 127] = 1.0

    common = {
        "iota_in": iota_np,
        "w2p": w2p,
        "w1": np.asarray(inputs["nn_w1"], np.float32),
        "b1": np.asarray(inputs["nn_b1"], np.float32)[:, None].copy(),
        "lin0aug": lin0aug,
        "xaug": xaug,
        "b2": np.asarray(inputs["nn_b2"], np.float32).reshape(DIM, DIM).copy(),
        "cr": np.asarray(inputs["conv_root"], np.float32),
        "cb": np.asarray(inputs["conv_bias"], np.float32)[:, None].copy(),
        "gwih": np.asarray(inputs["gru_w_ih"], np.float32),
        "gwhh": np.asarray(inputs["gru_w_hh"], np.float32),
        "gbr": (gih + ghh)[0:DIM, None].copy(),
        "gbz": (gih + ghh)[DIM:2 * DIM, None].copy(),
        "gbin": gih[2 * DIM:3 * DIM, None].copy(),
        "gbhn": ghh[2 * DIM:3 * DIM, None].copy(),
        "ohb": ohb_np.reshape(P, NBLK * B),
        "ohbt": ohbt_np.reshape(B, NBLK * P),
        "lwih": np.asarray(inputs["lstm_w_ih"], np.float32),
        "lwhh": np.asarray(inputs["lstm_w_hh"], np.float32),
        "lb": (np.asarray(inputs["lstm_b_ih"], np.float32)
               + np.asarray(inputs["lstm_b_hh"], np.float32))[:, None].copy(),
        "l1w": np.asarray(inputs["lin1_w"], np.float32),
        "l1b": np.asarray(inputs["lin1_b"], np.float32)[:, None].copy(),
        "l2w": np.asarray(inputs["lin2_w"], np.float32),
        "l2b": np.asarray(inputs["lin2_b"], np.float32)[:, None].copy(),
    }

    in_maps = []
    for c in range(NCORES):
        lo = c * BPC * EPB
        hi = (c + 1) * BPC * EPB
        m = dict(common)
        m["ea_t"] = np.ascontiguousarray(p_ea[lo:hi].T)
        m["src_i"] = np.ascontiguousarray(p_src[lo:hi].reshape(T, P).T)
        m["dst_f"] = np.ascontiguousarray(p_dstf[lo:hi].reshape(T, P).T)
        m["xaug_loc"] = np.ascontiguousarray(xaug[:, c * NLOC:(c + 1) * NLOC])
        m["deginv"] = np.broadcast_to(deginv_full[c * NLOC:(c + 1) * NLOC],
                                      (DIM, NLOC)).copy()
        in_maps.append(m)
    return in_maps, K


def kernel(**inputs):
    in_maps, K = _prep_inputs(inputs)
    if K not in _PROGRAM_CACHE:
        _PROGRAM_CACHE[K] = _build_program(K)
    nc = _PROGRAM_CACHE[K]
    res = bass_utils.run_bass_kernel_spmd(nc, in_maps, core_ids=list(range(NCORES)))
    return np.asarray(res.results[0]["y"][0], np.float32)
